# revision 13
# baseline (speedup 1.0000x reference)
"""Trainium2 Bass kernel for CrossAttention with LoRA.

Strategy: data-parallel over batch (B=8 -> 8 NeuronCores, one batch element
per core). No collectives. Per-core compute is a fully fused cross-attention:

  kT [C,S] = (Wf[:C].T row-tile col-slices) @ fT
  v  [S,C] = fT col-slices as lhsT @ Wf[C:].T          (natural layout)
  qT [C,T] = (Wq.T row-tile col-slices) @ xT           (x passed pre-transposed)
  per head h: sT[S,T] = kT_h.T-slices @ qT_h           (K=D=64)
              e = exp(sT/8) * causal_mask
              y_aug[65,T] = [v_h | ones].T @ e         (row 64 = softmax denom)
  normalize via reciprocal_approx_fast + selection-matrix broadcast matmul
  out [T,C] = yT col-slices as lhsT @ Wp.T (+ LoRA/bias terms)

All matmul operands are bf16 (PSUM accumulation in fp32). Pipeline notes:
 - Weight/input DMAs are contiguous row tiles split across the two HWDGE
   queues (SP: fT/Wfk/Wfv + SBUF bounces + stores; Act: esel/xT/Wq/Wp) so
   the PE starts ~2us in and operands always arrive ahead of use.
 - The q projection is interleaved with attention head pairs (software
   pipelined by one m-tile): the scalar engine's exp stream (~3us/head) is
   hidden under q-proj matmuls instead of serializing the attention phase.
 - Rowsums drain to fp32; one reciprocal_approx_fast + bf16 cast feeds the
   broadcast matmuls, keeping the PE gap at the attention->out-proj
   boundary small so the HAM clock stays at 2.4 GHz.
LoRA terms (rank 16) and biases fold into the same PSUM accumulation groups;
they are skipped at trace time when the corresponding host arrays are zero
(true for loralib-initialized B matrices and zero biases).
"""

import ml_dtypes
import numpy as np

import concourse.bass as bass  # noqa: F401  (bass types via bacc)
import concourse.mybir as mybir
import concourse.tile as tile
from concourse import bacc
from concourse.bass_utils import run_bass_kernel_spmd

B, T, S, C, H, D, R = 8, 1024, 256, 1024, 16, 64, 16
SCALING = 1.0 / 16.0
P = 128
KC = C // P  # 8 k-tiles over the embedding dim
MT = T // P  # 8 tiles over T
NCH = T // 512  # 2 psum chunks over T
F32 = mybir.dt.float32
F32R = mybir.dt.float32r
BF16 = mybir.dt.bfloat16
NPBF16 = ml_dtypes.bfloat16

_nc_cache: dict = {}


def _build(flags):
    has_lq, has_lf, has_lp, has_bq, has_bfk, has_bfv, has_bp = flags
    nc = bacc.Bacc("TRN2", target_bir_lowering=False, debug=False)

    xT = nc.declare_dram_parameter("xT", [C, T], BF16, isOutput=False)
    fT = nc.declare_dram_parameter("fT", [C, S], BF16, isOutput=False)
    WqT = nc.declare_dram_parameter("WqT", [C, C], BF16, isOutput=False)
    WfkT = nc.declare_dram_parameter("WfkT", [C, C], BF16, isOutput=False)
    WfvT = nc.declare_dram_parameter("WfvT", [C, C], BF16, isOutput=False)
    WpT = nc.declare_dram_parameter("WpT", [C, C], BF16, isOutput=False)
    mask = nc.declare_dram_parameter("mask", [P, 384], BF16, isOutput=False)
    Esel = nc.declare_dram_parameter("Esel", [H, C], F32R, isOutput=False)
    if has_lq:
        AqT = nc.declare_dram_parameter("AqT", [C, R], BF16, isOutput=False)
        BqTs = nc.declare_dram_parameter("BqTs", [R, C], BF16, isOutput=False)
    if has_lf:
        AfT = nc.declare_dram_parameter("AfT", [C, R], BF16, isOutput=False)
        BfkTs = nc.declare_dram_parameter("BfkTs", [R, C], BF16, isOutput=False)
        BfvTs = nc.declare_dram_parameter("BfvTs", [R, C], BF16, isOutput=False)
    if has_lp:
        ApT = nc.declare_dram_parameter("ApT", [C, R], BF16, isOutput=False)
        BpTs = nc.declare_dram_parameter("BpTs", [R, C], BF16, isOutput=False)
    if has_bq:
        bq_pp = nc.declare_dram_parameter("bq_pp", [P, KC], F32, isOutput=False)
    if has_bfk:
        bfk_pp = nc.declare_dram_parameter("bfk_pp", [P, KC], F32, isOutput=False)
    if has_bfv:
        bfv_row = nc.declare_dram_parameter("bfv_row", [1, C], BF16, isOutput=False)
    if has_bp:
        bp_row = nc.declare_dram_parameter("bp_row", [1, C], BF16, isOutput=False)
    out = nc.declare_dram_parameter("out", [T, C], F32, isOutput=True)
    import os
    DBG = bool(os.environ.get("KERNEL_DEBUG"))
    if DBG:
        dbg_k0 = nc.declare_dram_parameter("dbg_k0", [P, S], F32, isOutput=True)
        dbg_q0 = nc.declare_dram_parameter("dbg_q0", [P, T], F32, isOutput=True)
        dbg_v0 = nc.declare_dram_parameter("dbg_v0", [P, H * (D + 1)], F32, isOutput=True)
        dbg_r = nc.declare_dram_parameter("dbg_r", [H, T], F32, isOutput=True)
        dbg_y0 = nc.declare_dram_parameter("dbg_y0", [P, T], F32, isOutput=True)
        dbg_rec = nc.declare_dram_parameter("dbg_rec", [H, T], F32, isOutput=True)
        dbg_rb = nc.declare_dram_parameter("dbg_rb", [P, T], F32, isOutput=True)
        dbg_yu = nc.declare_dram_parameter("dbg_yu", [P, T], F32, isOutput=True)

    # 3D row-tile views for contiguous tiled DMA
    xT3 = xT.rearrange("(ko p) t -> ko p t", p=P)
    fT3 = fT.rearrange("(ko p) s -> ko p s", p=P)
    WqT3 = WqT.rearrange("(ko p) c -> ko p c", p=P)
    WfkT3 = WfkT.rearrange("(ko p) c -> ko p c", p=P)
    WfvT3 = WfvT.rearrange("(ko p) c -> ko p c", p=P)
    WpT3 = WpT.rearrange("(ko p) c -> ko p c", p=P)

    def c512(i):
        return slice(i * 512, (i + 1) * 512)

    def mP(m):
        return slice(m * P, (m + 1) * P)

    with tile.TileContext(nc) as tc:
        with (
            tc.tile_pool(name="big", bufs=2) as big,      # xT merged tiles
            tc.tile_pool(name="ypool", bufs=8) as ypool,  # yTr tiles (live while
            # q-proj still reads xT — the interleave forbids sharing big's bufs)
            tc.tile_pool(name="qpool", bufs=8) as qpool,   # qT tiles
            tc.tile_pool(name="wts4", bufs=4) as wts4p,   # wfk/wfv merged tiles
            tc.tile_pool(name="wts8", bufs=2) as wts8p,   # wq/wp merged tiles
            tc.tile_pool(name="small", bufs=1) as small,   # long-lived small tiles
            tc.tile_pool(name="expp", bufs=6) as expp,    # per-head exp tiles
            tc.tile_pool(name="stg", bufs=3) as stg,      # bf16 head stage
            tc.tile_pool(name="stgf", bufs=2) as stgf,    # f32 rowsum stage
            tc.tile_pool(name="ostg", bufs=3) as ostg,    # out staging
            tc.tile_pool(name="psA", bufs=4, space="PSUM") as psA,
            tc.tile_pool(name="psB", bufs=2, space="PSUM") as psB,
        ):
            # ---- SP queue: k/v-projection operands first. DMA dispatch costs
            # the issuing engine ~650ns per instruction, so loads are merged
            # into multi-tile transfers (2KB/partition descriptor runs).
            fTa = small.tile([P, KC, S], BF16, tag="fTa", name="fTa")
            nc.sync.dma_start(fTa[:], fT.rearrange("(ko p) s -> p ko s", p=P))
            fTs = [fTa[:, k, :] for k in range(KC)]
            wfk4 = [wts4p.tile([P, 4, C], BF16, tag="wts4", name=f"wfk4_{g}") for g in range(2)]
            for g in range(2):
                nc.sync.dma_start(
                    wfk4[g][:], WfkT.rearrange("(ko p) c -> p ko c", p=P)[:, 4 * g:4 * g + 4, :])
            wfk = [wfk4[k // 4][:, k % 4, :] for k in range(KC)]
            wfv4 = [wts4p.tile([P, 4, C], BF16, tag="wts4", name=f"wfv4_{g}") for g in range(2)]
            for g in range(2):
                nc.sync.dma_start(
                    wfv4[g][:], WfvT.rearrange("(ko p) c -> p ko c", p=P)[:, 4 * g:4 * g + 4, :])
            wfv = [wfv4[k // 4][:, k % 4, :] for k in range(KC)]
            mask_sb = small.tile([P, 384], BF16, tag="mask", name="mask_sb")
            nc.sync.dma_start(mask_sb[:], mask[:, :])
            # ---- Act queue: esel + q/out-projection operands, issued up front --
            esel_sb = small.tile([H, C], F32R, tag="esel", name="esel_sb")
            nc.scalar.dma_start(esel_sb[:], Esel[:, :])
            xT4 = [big.tile([P, 4, T], BF16, tag="big", name=f"xT4_{g}") for g in range(2)]
            for g in range(2):
                nc.scalar.dma_start(
                    xT4[g][:], xT.rearrange("(ko p) t -> p ko t", p=P)[:, 4 * g:4 * g + 4, :])
            xTs = [xT4[k // 4][:, k % 4, :] for k in range(KC)]
            wqa = wts8p.tile([P, KC, C], BF16, tag="wts8", name="wqa")
            nc.scalar.dma_start(wqa[:], WqT.rearrange("(ko p) c -> p ko c", p=P))
            wq = [wqa[:, k, :] for k in range(KC)]
            wpa = wts8p.tile([P, KC, C], BF16, tag="wts8", name="wpa")
            nc.scalar.dma_start(wpa[:], WpT.rearrange("(ko p) c -> p ko c", p=P))
            wp = [wpa[:, k, :] for k in range(KC)]
            # ---- small conditional loads (SP) ----------------------------------
            if has_lq:
                aq_sb = small.tile([P, KC, R], BF16, tag="aq", name="aq_sb")
                nc.sync.dma_start(aq_sb[:], AqT.rearrange("(ko p) r -> p ko r", p=P))
                bqs_sb = small.tile([R, C], BF16, tag="bqs", name="bqs_sb")
                nc.sync.dma_start(bqs_sb[:], BqTs[:, :])
            if has_lf:
                af_sb = small.tile([P, KC, R], BF16, tag="af", name="af_sb")
                nc.sync.dma_start(af_sb[:], AfT.rearrange("(ko p) r -> p ko r", p=P))
                bfks_sb = small.tile([R, C], BF16, tag="bfks", name="bfks_sb")
                nc.sync.dma_start(bfks_sb[:], BfkTs[:, :])
                bfvs_sb = small.tile([R, C], BF16, tag="bfvs", name="bfvs_sb")
                nc.sync.dma_start(bfvs_sb[:], BfvTs[:, :])
            if has_lp:
                ap_sb = small.tile([P, KC, R], BF16, tag="ap", name="ap_sb")
                nc.sync.dma_start(ap_sb[:], ApT.rearrange("(ko p) r -> p ko r", p=P))
                bps_sb = small.tile([R, C], BF16, tag="bps", name="bps_sb")
                nc.sync.dma_start(bps_sb[:], BpTs[:, :])
            if has_bq:
                bq_sb = small.tile([P, KC], F32, tag="bq", name="bq_sb")
                nc.sync.dma_start(bq_sb[:], bq_pp[:, :])
            if has_bfk:
                bfk_sb = small.tile([P, KC], F32, tag="bfk", name="bfk_sb")
                nc.sync.dma_start(bfk_sb[:], bfk_pp[:, :])
            if has_bfv:
                bfv_sb = small.tile([1, C], BF16, tag="bfv", name="bfv_sb")
                nc.sync.dma_start(bfv_sb[:], bfv_row[:, :])
            if has_bp:
                bp_sb = small.tile([1, C], BF16, tag="bp", name="bp_sb")
                nc.sync.dma_start(bp_sb[:], bp_row[:, :])
            ones1 = None
            if has_bfv or has_bp:
                ones1 = small.tile([1, P], BF16, tag="ones1", name="ones1")
                nc.scalar.activation(
                    ones1[:], mask_sb[0:1, 0:P],
                    mybir.ActivationFunctionType.Copy, bias=1.0, scale=0.0,
                )

            # ---- LoRA u-vector for kv (needs only fT) --------------------------
            if has_lf:
                ufs = psB.tile([P, T], F32, tag="y", name="uf_ps")
                for k in range(KC):
                    nc.tensor.matmul(
                        ufs[:R, :S], af_sb[:, k, :], fTs[k][:],
                        start=(k == 0), stop=(k == KC - 1),
                    )
                uf_sb = small.tile([R, S], BF16, tag="uf", name="uf_sb")
                nc.scalar.copy(uf_sb[:], ufs[:R, :S])

            # ---- k projection: kT [C, S] ---------------------------------------
            kTs = [small.tile([P, S], BF16, tag=f"kT{m}", name=f"kT{m}") for m in range(KC)]
            for m in range(KC):
                ps = psA.tile([P, S], F32, tag="mm", name=f"k_ps{m}")
                for k in range(KC):
                    nc.tensor.matmul(
                        ps[:], wfk[k][:, mP(m)], fTs[k][:],
                        start=(k == 0), stop=(k == KC - 1 and not has_lf),
                    )
                if has_lf:
                    nc.tensor.matmul(
                        ps[:], bfks_sb[:, mP(m)], uf_sb[:],
                        start=False, stop=True,
                    )
                if has_bfk:
                    nc.scalar.activation(
                        kTs[m][:], ps[:], mybir.ActivationFunctionType.Identity,
                        bias=bfk_sb[:, m:m + 1], scale=1.0,
                    )
                else:
                    nc.vector.tensor_copy(kTs[m][:], ps[:])

            # ---- v projection: v_aug [S, H, D+1] (ones col appended) -----------
            v_aug = [
                small.tile([P, H, D + 1], BF16, tag=f"vaug{s2}", name=f"vaug{s2}")
                for s2 in range(2)
            ]
            for s2 in range(2):
                # ones column via ACT const-fill (mask_sb is a known-finite input)
                nc.scalar.activation(
                    v_aug[s2][:, :, D], mask_sb[:, 0:H],
                    mybir.ActivationFunctionType.Copy, bias=1.0, scale=0.0,
                )
                for ch in range(NCH):
                    ps = psA.tile([P, 8, D], F32, tag="mm", name=f"v_ps{s2}_{ch}")
                    nmm = KC + (1 if has_lf else 0) + (1 if has_bfv else 0)
                    i = 0
                    for k in range(KC):
                        i += 1
                        nc.tensor.matmul(
                            ps[:], fTs[k][:, s2 * P:(s2 + 1) * P],
                            wfv[k][:, c512(ch)],
                            start=(i == 1), stop=(i == nmm),
                        )
                    if has_lf:
                        i += 1
                        nc.tensor.matmul(
                            ps[:], uf_sb[:, s2 * P:(s2 + 1) * P],
                            bfvs_sb[:, c512(ch)], start=False, stop=(i == nmm),
                        )
                    if has_bfv:
                        i += 1
                        nc.tensor.matmul(
                            ps[:], ones1[:], bfv_sb[:, c512(ch)],
                            start=False, stop=(i == nmm),
                        )
                    nc.vector.tensor_copy(
                        v_aug[s2][:, ch * 8:(ch + 1) * 8, 0:D], ps[:]
                    )

            # ---- LoRA u-vector for q (needs xT) --------------------------------
            if has_lq:
                ups = psB.tile([P, T], F32, tag="y", name="uq_ps")
                for ch in range(NCH):
                    for k in range(KC):
                        nc.tensor.matmul(
                            ups[:R, c512(ch)], aq_sb[:, k, :], xTs[k][:, c512(ch)],
                            start=(k == 0), stop=(k == KC - 1),
                        )
                uq_sb = small.tile([R, T], BF16, tag="uq", name="uq_sb")
                nc.scalar.copy(uq_sb[:], ups[:R, :])

            # ---- interleaved q projection + attention --------------------------
            # qproj(m) then heads(2(m-1), 2(m-1)+1): the one-m software pipeline
            # hides the qT drain latency and keeps the PE fed while the scalar
            # engine runs the exp stream.
            qTs = [qpool.tile([P, T], BF16, tag="qT", name=f"qT{m}") for m in range(MT)]
            yTr = [ypool.tile([P, T], BF16, tag="y", name=f"yTr{p}") for p in range(KC)]
            r_f32 = small.tile([H, T], F32, tag="rsum", name="r_f32")

            def qhalf(j):
                m, ch = j // 2, j % 2
                ps = psA.tile([P, 512], F32, tag="mm", name=f"q_ps{m}_{ch}")
                for k in range(KC):
                    nc.tensor.matmul(
                        ps[:], wq[k][:, mP(m)], xTs[k][:, c512(ch)],
                        start=(k == 0), stop=(k == KC - 1 and not has_lq),
                    )
                if has_lq:
                    nc.tensor.matmul(
                        ps[:], bqs_sb[:, mP(m)], uq_sb[:, c512(ch)],
                        start=False, stop=True,
                    )
                if has_bq:
                    nc.scalar.activation(
                        qTs[m][:, c512(ch)], ps[:],
                        mybir.ActivationFunctionType.Identity,
                        bias=bq_sb[:, m:m + 1], scale=1.0,
                    )
                else:
                    nc.vector.tensor_copy(qTs[m][:, c512(ch)], ps[:])

            es_all = {}

            def scores(h):
                m, off = h // 2, (h % 2) * D
                kt_h = kTs[m][off:off + D, :]
                qt_h = qTs[m][off:off + D, :]
                es = [expp.tile([P, T], BF16, tag="exp", name=f"e{h}_{s2}") for s2 in range(2)]
                es_all[h] = es
                for s2 in range(2):
                    for ch in range(NCH):
                        ps = psA.tile([P, 512], F32, tag="mm", name=f"s_ps{h}_{s2}_{ch}")
                        nc.tensor.matmul(
                            ps[:], kt_h[:, s2 * P:(s2 + 1) * P], qt_h[:, c512(ch)],
                            start=True, stop=True,
                        )
                        nc.scalar.activation(
                            es[s2][:, c512(ch)], ps[:],
                            mybir.ActivationFunctionType.Exp, scale=0.125,
                        )
                nc.vector.tensor_mul(es[0][:, 0:P], es[0][:, 0:P], mask_sb[:, 0:P])
                nc.vector.tensor_mul(es[1][:, 0:S], es[1][:, 0:S], mask_sb[:, P:384])

            def attnv(h):
                m, off = h // 2, (h % 2) * D
                es = es_all.pop(h)
                psy = psB.tile([P, T], F32, tag="y", name=f"y_ps{h}")
                for ch in range(NCH):
                    for s2 in range(2):
                        nc.tensor.matmul(
                            psy[:D + 1, c512(ch)], v_aug[s2][:, h, :],
                            es[s2][:, c512(ch)], start=(s2 == 0), stop=(s2 == 1),
                        )
                # drain: rows 0:64 -> stacked yTr (cast fp32->bf16), row 64
                # (rowsum) -> f32 stage -> r_f32. DMA can't read PSUM and
                # compute engines can't shift partitions, so odd heads bounce
                # through an SBUF staging tile + SBUF->SBUF DMA.
                stf = stgf.tile([D + 1, T], F32, tag="fstage", name=f"stf{h}")
                nc.vector.tensor_copy(stf[D:D + 1, :], psy[D:D + 1, :])
                nc.sync.dma_start(r_f32[h:h + 1, :], stf[D:D + 1, :])
                if off == 0:
                    nc.vector.tensor_copy(yTr[m][0:D, :], psy[0:D, :])
                else:
                    st = stg.tile([P, T], BF16, tag="hstage", name=f"st{h}")
                    nc.vector.tensor_copy(st[0:D, :], psy[0:D, :])
                    nc.sync.dma_start(yTr[m][off:off + D, :], st[0:D, :])

            # one head per iteration, with the next head's scores and a q-proj
            # half-tile emitted between a head's scores and its attnv so the
            # scalar engine's exp latency stays off the PE critical path.
            qhalf(0)
            qhalf(1)
            scores(0)
            for h in range(H):
                if h + 2 < 2 * MT:
                    qhalf(h + 2)
                if h + 1 < H:
                    scores(h + 1)
                attnv(h)

            # ---- normalize: yTr *= broadcast(1/rowsum) -------------------------
            recf = small.tile([H, T], F32, tag="recf", name="recf")
            nc.vector.reciprocal_approx_fast(recf[:], r_f32[:])
            rec = small.tile([H, T], F32R, tag="rec", name="rec")
            nc.vector.tensor_copy(rec[:], recf[:])
            for p in range(KC):
                rb = psB.tile([P, T], F32, tag="y", name=f"rb{p}")
                for ch in range(NCH):
                    nc.tensor.matmul(
                        rb[:, c512(ch)], esel_sb[:, mP(p)],
                        rec[:, c512(ch)], start=True, stop=True,
                    )
                if DBG and p == 0:
                    dstage0 = small.tile([P, T], F32, tag="dstage0", name="dstage0")
                    nc.vector.tensor_copy(dstage0[:], rb[:])
                    nc.sync.dma_start(dbg_rb[:, :], dstage0[:])
                    dstage1 = small.tile([P, T], F32, tag="dstage1", name="dstage1")
                    nc.vector.tensor_copy(dstage1[:], yTr[p][:])
                    nc.sync.dma_start(dbg_yu[:, :], dstage1[:])
                nc.vector.tensor_mul(yTr[p][:], yTr[p][:], rb[:])

            if DBG:
                dstage = small.tile([P, T], F32, tag="dstage", name="dstage")
                nc.vector.tensor_copy(dstage[:, 0:S], kTs[0][:])
                nc.sync.dma_start(dbg_k0[:, :], dstage[:, 0:S])
                nc.vector.tensor_copy(dstage[:], qTs[0][:])
                nc.sync.dma_start(dbg_q0[:, :], dstage[:])
                vstage = small.tile([P, H, D + 1], F32, tag="vstage", name="vstage")
                nc.vector.tensor_copy(vstage[:], v_aug[0][:])
                nc.sync.dma_start(dbg_v0[:, :], vstage[:])
                nc.vector.tensor_copy(dstage[0:H, :], r_f32[:])
                nc.sync.dma_start(dbg_r[:, :], dstage[0:H, :])
                nc.vector.tensor_copy(dstage[:], yTr[0][:])
                nc.sync.dma_start(dbg_y0[:, :], dstage[:])
                nc.vector.tensor_copy(dstage[0:H, :], rec[:])
                nc.sync.dma_start(dbg_rec[:, :], dstage[0:H, :])

            # ---- output projection: out [T, C] ---------------------------------
            if has_lp:
                upsd = psB.tile([P, T], F32, tag="y", name="up_ps")
                for ch in range(NCH):
                    for k in range(KC):
                        nc.tensor.matmul(
                            upsd[:R, c512(ch)], ap_sb[:, k, :], yTr[k][:, c512(ch)],
                            start=(k == 0), stop=(k == KC - 1),
                        )
                up_sb = small.tile([R, T], BF16, tag="up", name="up_sb")
                nc.scalar.copy(up_sb[:], upsd[:R, :])
            for m in range(MT):
                for ch in range(NCH):
                    ps = psA.tile([P, 512], F32, tag="mm", name=f"o_ps{m}_{ch}")
                    nmm = KC + (1 if has_lp else 0) + (1 if has_bp else 0)
                    i = 0
                    for k in range(KC):
                        i += 1
                        nc.tensor.matmul(
                            ps[:], yTr[k][:, mP(m)], wp[k][:, c512(ch)],
                            start=(i == 1), stop=(i == nmm),
                        )
                    if has_lp:
                        i += 1
                        nc.tensor.matmul(
                            ps[:], up_sb[:, mP(m)], bps_sb[:, c512(ch)],
                            start=False, stop=(i == nmm),
                        )
                    if has_bp:
                        i += 1
                        nc.tensor.matmul(
                            ps[:], ones1[:], bp_sb[:, c512(ch)],
                            start=False, stop=(i == nmm),
                        )
                    ost = ostg.tile([P, 512], F32, tag="ostage", name=f"ost{m}_{ch}")
                    nc.vector.tensor_copy(ost[:], ps[:])
                    nc.sync.dma_start(out[mP(m), c512(ch)], ost[:])

    nc.finalize()
    return nc


def _bf(a):
    return np.ascontiguousarray(np.asarray(a, np.float32).astype(NPBF16))


def _host_prep(x, feature, Wq, bq, Aq, Bq, Wf, bf, Af, Bf, Wp, bp, Ap, Bp):
    f32 = np.float32
    flags = (
        bool(np.any(Bq)), bool(np.any(Bf)), bool(np.any(Bp)),
        bool(np.any(bq)), bool(np.any(bf[:C])), bool(np.any(bf[C:])),
        bool(np.any(bp)),
    )
    shared = {
        "WqT": _bf(np.asarray(Wq, f32).T),
        "WfkT": _bf(np.asarray(Wf[:C], f32).T),
        "WfvT": _bf(np.asarray(Wf[C:], f32).T),
        "WpT": _bf(np.asarray(Wp, f32).T),
    }
    i = np.arange(P)[:, None]
    j = np.arange(384)[None, :]
    m0 = (j[:, :P] >= i).astype(f32)
    m1 = ((j[:, P:384] - P) >= (P + i)).astype(f32)
    shared["mask"] = _bf(np.concatenate([m0, m1], axis=1))
    hsel = np.arange(H)[:, None]
    col = np.arange(C)[None, :]
    shared["Esel"] = np.ascontiguousarray((hsel == col // D).astype(f32))
    has_lq, has_lf, has_lp, has_bq, has_bfk, has_bfv, has_bp = flags
    if has_lq:
        shared["AqT"] = _bf(np.asarray(Aq, f32).T)
        shared["BqTs"] = _bf(np.asarray(Bq, f32).T * SCALING)
    if has_lf:
        shared["AfT"] = _bf(np.asarray(Af, f32).T)
        shared["BfkTs"] = _bf(np.asarray(Bf[:C], f32).T * SCALING)
        shared["BfvTs"] = _bf(np.asarray(Bf[C:], f32).T * SCALING)
    if has_lp:
        shared["ApT"] = _bf(np.asarray(Ap, f32).T)
        shared["BpTs"] = _bf(np.asarray(Bp, f32).T * SCALING)
    if has_bq:
        shared["bq_pp"] = np.ascontiguousarray(np.asarray(bq, f32).reshape(KC, P).T)
    if has_bfk:
        shared["bfk_pp"] = np.ascontiguousarray(np.asarray(bf[:C], f32).reshape(KC, P).T)
    if has_bfv:
        shared["bfv_row"] = _bf(np.asarray(bf[C:], f32).reshape(1, C))
    if has_bp:
        shared["bp_row"] = _bf(np.asarray(bp, f32).reshape(1, C))

    in_maps = []
    for b in range(B):
        m = dict(shared)
        m["xT"] = _bf(np.asarray(x[b], f32).T)
        m["fT"] = _bf(np.asarray(feature[b], f32).T)
        in_maps.append(m)
    return flags, in_maps


def _run(inputs, trace=False, **spmd_kwargs):
    flags, in_maps = _host_prep(**inputs)
    nc = _nc_cache.get(flags)
    if nc is None:
        nc = _build(flags)
        _nc_cache[flags] = nc
    res = run_bass_kernel_spmd(
        nc, in_maps, core_ids=list(range(B)), trace=trace, **spmd_kwargs
    )
    out = np.stack([res.results[b]["out"] for b in range(B)], axis=0)
    return out, res


def kernel(**inputs):
    out, _ = _run(inputs, trace=False)
    return out
